# revision 39
# baseline (speedup 1.0000x reference)
"""Trainium2 Bass kernel for nn_EstimatorQNN (18-qubit QNN, batch 16), v4.

Math: each <Z_i> is an exact 5-qubit light-cone sim over wires
{i-2..i+2}. v2 reductions (fused RX+RY layer-1 gates, CZ-conjugation
sigma masks, chi/gauge cancellation, layer-3 fold into the
measurement) still apply; v4 adds the CONTROL-BIT FACTORIZATION:

  Window wires i-2 and i+2 (bits 0 and 4) are diagonal controls for
  the whole circuit after layer 1 - they only enter through the
  CZ-conjugation signs sigma_1 = (-1)^(b0+b2), sigma_3 = (-1)^(b2+b4).
  Hence EXACTLY
      E = sum_{b0,b4} |f0(b0)|^2 |f4(b4)|^2 E(b0,b4),
  with f_s = G_s (1, -i) the post-layer-1 single-wire amplitudes. The
  simulated state shrinks to bits 1..3 (8 amps, 16 re/im cols), layer-1
  needs only 3 gates (on half-width ops), and the b0/b4 gates become
  per-branch weights shipped to the host (computed on-device as free
  1-col ops; the host does only the 4-term weighted average).

Layout per core: 2 samples x 18 windows = 36 sims; rows r (v4=0) and
64+r (v4=1), dead band 36-63 (partition bases must be 0/32/64/96).
State cols pre-replicate: 16 = (b3, b2, b1, ri) outer->inner; after
the v0 replication: 32 = (b3, v0, b2, b1, ri), so b3 stays the GLOBAL
top bit and layer-2 slot 3 keeps its 2-op reversed-view form (3D AP
limit: <=2 free dims).

Trig: ONE Activation op produces both tables - input angle block is
[a | a - pi] so Sin(0.5 in + pi/2) = cos(a/2) | sin(a/2) (table-exact;
rel err ~9e-5 = the complex64 reference's own noise floor).

Layer-2 ordering: slots 2 and 3 run BEFORE the v0 replication (their
conjugation signs don't involve v0 and the rotations commute), on
half-width operands; only slot 1 runs on the replicated state.

v5: the post-layer-1 state is a RANK-1 PRODUCT f3 (x) f2 (x) f1 with
f_s = G_s (1,-i) (phase included), and every component is a
per-partition scalar already on hand (cw*cx etc). The state is
therefore CONSTRUCTED directly - the textbook product-state build -
as 1-col scalar ops (free, pipelined), writing both v0 blocks of the
32-col state; the 12 wide layer-1 gate ops, the phase-table input and
the 2 replication ops all disappear. Only the three layer-2 rotations
(non-product) and the measurement remain as wide ops.

Timing structure (CoreSim, 5014ns total): startup 500 + input-DMA sem
~950 + ACT op -> DVE chain from 1591: ~60 free 1-col scalar ops
(trig products, w3 folds, branch weights, product-state build) then
14 serial wide ops (layer-2 j=2: 3, j=3: 2, j=1: 3, measurement: 6)
at the 61 + W_out/2 ns width law; sigma masks arrive via Pool piece 2
(first consumed by the j=2 cross-term op); then the fixed out-DMA
tail (~470 dispatch + 1717 DMACopy visit).

The Bass-emitted prologue (const-AP memsets + all-engine barrier) and
epilogue barrier are stripped post-build; program end is ordered after
the output DMA by an SP wait on its completion semaphore (without any
ordering the out-DMA races program end on real HW).
"""

import sys

sys.path.insert(0, "/opt/trn_rl_repo")

import numpy as np

import concourse.bass as bass
import concourse.mybir as mybir
from concourse.bass_utils import run_bass_kernel_spmd

NQ = 18
BATCH = 16
NCORES = 8
SPB = BATCH // NCORES  # samples per core
ROWS = SPB * NQ  # 36 sims per core (rows 0..35 and 64..99)
HI = 64  # partition base of the v4=1 row group
NROWS = HI + ROWS  # 100 partitions used
W = 5  # window width
NA8 = 8  # branch amplitudes (b1..b3)
NST = 2 * NA8  # 16 state cols pre-replicate
NST2 = 2 * NST  # 32 cols after v0 replication
NANG = 14  # angle cols: 5 x | 5 w1 | 3 w2(slots 1-3) | 1 w3
NANGA = 2 * NANG  # input angle block: [a | a - pi]
NK = 12  # const-scalar cols
NRES = 8  # out cols: res0/res1 per v0 variant, then w00 w01 w40 w41
# input cols: [angles(28) | consts(12) || phase(16) | sgn1|sgn2|sgn3(16 ea)]
C_ANG = 0
C_K = NANGA
C_SG = NANGA + NK  # masks ride the single SP input DMA too
CC = C_SG + 3 * NST  # all masks 16-wide (state is built replicated)
C_AK = CC  # one flat input DMA carries everything

F32 = mybir.dt.float32
ALU = mybir.AluOpType

HALFPI = 1.5707963267948966
KVALS = [0.0, 0.0, HALFPI, 0.0, 0.0, 0.0, 1.0, -8.0, -4.0, 2.0, -1.0, 0.0]
(_KI_A, _KI_B, KI_HPI, _KI_CA, _KI_CB, _KI_C2, KI_ONE, KI_NEG8,
 KI_NEG4, KI_TWO, KI_NEGONE, _KI_PAD) = range(NK)


def _const_rows() -> tuple[np.ndarray, np.ndarray]:
    """(row_lo, row_hi): cols C_K..CC for the v4=0 / v4=1 row groups."""
    # phase over branch bits: col = ri + 2 b1 + 4 b2 + 8 b3
    a = np.arange(NA8)
    b1 = a & 1
    b2 = (a >> 1) & 1
    b3 = (a >> 2) & 1
    # masks over post-replication amps: idx = b1 + 2 b2 + 4 v0 + 8 b3
    m = np.arange(16)
    mb1 = m & 1
    mb2 = (m >> 1) & 1
    mv0 = (m >> 2) & 1
    mb3 = (m >> 3) & 1
    sgn1 = (-1.0) ** (mv0 + mb2)
    sgn2 = (-1.0) ** (mb1 + mb3)
    out = []
    for v4 in (0, 1):
        sgn3 = (1.0 - 2.0 * mb3) * ((-1.0) ** (mb2 + v4))
        out.append(np.concatenate(
            [KVALS, sgn1, sgn2, sgn3]).astype(np.float32))
    return out[0], out[1]


def _angle_table(x: np.ndarray, params: np.ndarray) -> np.ndarray:
    """[BATCH, NQ, NANGA] per-sim angles (0 for padded window slots)."""
    w1 = params[NQ:2 * NQ]
    w2 = params[2 * NQ:3 * NQ]
    w3 = params[3 * NQ:]
    A = np.zeros((BATCH, NQ, NANGA), np.float32)
    for i in range(NQ):
        for k in range(W):
            j = i - 2 + k
            if 0 <= j < NQ:
                A[:, i, k] = x[:, j]
                A[:, i, W + k] = w1[j]
        for m in range(3):  # L2 slots 1,2,3 -> wires i-1, i, i+1
            j = i - 1 + m
            if 0 <= j < NQ:
                A[:, i, 2 * W + m] = w2[j]
        A[:, i, 13] = w3[i]
    # cols 14..27: a - pi -> one ACT Sin op yields cos(a/2) AND sin(a/2)
    A[:, :, NANG:] = A[:, :, :NANG] - np.pi
    return A


def _bv(ap, ncols: int, k: int, b: int):
    """Bit-k == b view of an [p, ncols] re/im-interleaved state, bit
    order (.., k+1, k, .., ri): free dims [hi, lo] after slicing."""
    lo = 2 << k
    hi = ncols // (2 * lo)
    v = ap.rearrange("p (h c m) -> p h c m", h=hi, c=2, m=lo)
    return v[:, :, b, :]


def _build_nc(detect_races: bool = True) -> bass.Bass:
    nc = bass.Bass(detect_race_conditions=detect_races)
    inp = nc.dram_tensor("inp", [NROWS, CC], F32, kind="ExternalInput")
    outp = nc.dram_tensor("outp", [NROWS, NRES], F32, kind="ExternalOutput")

    with (
        nc.sbuf_tensor([128, CC], F32) as IN,
        nc.sbuf_tensor([128, 2 * NANG], F32) as TRG,  # cos | sin
        nc.sbuf_tensor([128, 40], F32) as PR,
        nc.sbuf_tensor([128, NST], F32) as ST16,
        nc.sbuf_tensor([128, NST2], F32) as ST,  # replicated state
        nc.sbuf_tensor([128, NST2], F32) as T,
        nc.sbuf_tensor([128, NST2], F32) as SCR,
        nc.sbuf_tensor([128, NRES], F32) as RES,
        nc.semaphore() as act_sem,
        nc.semaphore() as dma_sem,
        nc.semaphore() as dve_sem,
        nc.Block() as block,
    ):
        ang = IN[0:NROWS, C_ANG:C_ANG + NANGA]

        def K(i):
            return IN[0:NROWS, C_K + i:C_K + i + 1]

        st16 = ST16[0:NROWS, 0:NST]  # built by the rank-1 product
        st32 = ST[0:NROWS, 0:NST2]

        def sgn(j):  # layer-2 sign mask for slot j (re/im via bc)
            c0 = C_SG + (j - 1) * NST
            v = IN[0:NROWS, c0:c0 + NST].rearrange(
                "p (a b) -> p a b", a=NST, b=1)
            return v.broadcast_to([NROWS, NST, 2])

        trg = TRG[0:NROWS, 0:2 * NANG]
        t32 = T[0:NROWS, 0:NST2]

        def prc(i):
            return PR[0:NROWS, i:i + 1]

        def csc(i):
            return TRG[0:NROWS, i:i + 1]

        def sac(i):
            return TRG[0:NROWS, NANG + i:NANG + i + 1]

        def rcol(i):
            return RES[0:NROWS, i:i + 1]

        @block.sync
        def _(sync):
            sync.dma_start(
                out=IN[0:NROWS, 0:C_AK], in_=inp[:, 0:C_AK]).then_inc(
                dma_sem, 16)
            sync.dma_start(
                out=outp[:, :], in_=RES[0:NROWS, 0:NRES])._wait_ge(
                dve_sem, 1).then_inc(dma_sem, 16)
            sync.wait_ge(dma_sem, 32)

        @block.scalar
        def _(scalar):
            scalar.activation(
                trg, ang, mybir.ActivationFunctionType.Sin,
                bias=K(KI_HPI), scale=0.5)._wait_ge(dma_sem, 16).then_inc(
                act_sem, 1)

        @block.vector
        def _(vector):
            stt = vector.scalar_tensor_tensor
            ts = vector.tensor_scalar
            tsm = vector.tensor_scalar_mul

            # --- scalar products: PR[0:5]=cw1*cx, PR[5:10]=cw1*sx,
            # as ten 1-col ops (pipeline-hidden; on real HW they sit
            # inside the piece-2 DMA wait) ---
            first = True
            for k in range(W):
                op = vector.tensor_scalar_mul(prc(k), csc(W + k), csc(k))
                if first:
                    op._wait_ge(act_sem, 1)
                    first = False
                vector.tensor_scalar_mul(prc(W + k), csc(W + k), sac(k))
            # m2s = -4 sin(w3); n2c = -2 cos(w3) (1-col ops: free)
            stt(prc(10), sac(13), K(KI_NEG8), csc(13), ALU.mult, ALU.mult)
            stt(prc(11), csc(13), K(KI_NEG4), csc(13), ALU.mult, ALU.mult)
            vector.tensor_scalar_add(prc(11), prc(11), K(KI_TWO))

            # --- control-slot weights |f_s(b)|^2 into RES[4:8] (free
            # 1-col ops; f_s = G_s (1,-i), |f(0)|^2 = (cw cx)^2+(sw sx)^2,
            # |f(1)|^2 = (sw cx)^2 + (cw sx)^2, slots s=0 and 4) ---
            for w_i, s in ((0, 0), (1, 4)):
                u_ss = prc(12 + 2 * w_i)      # sw*sx
                u_sc = prc(13 + 2 * w_i)      # sw*cx
                stt(u_ss, sac(W + s), K(KI_ONE), sac(s), ALU.mult, ALU.mult)
                stt(u_sc, sac(W + s), K(KI_ONE), csc(s), ALU.mult, ALU.mult)
                qa = prc(16 + 2 * w_i)
                qb = prc(17 + 2 * w_i)
                stt(qa, prc(s), K(KI_ONE), prc(s), ALU.mult, ALU.mult)
                stt(qb, u_ss, K(KI_ONE), u_ss, ALU.mult, ALU.mult)
                stt(rcol(4 + 2 * w_i), qa, K(KI_ONE), qb, ALU.mult, ALU.add)
                stt(qa, u_sc, K(KI_ONE), u_sc, ALU.mult, ALU.mult)
                stt(qb, prc(W + s), K(KI_ONE), prc(W + s), ALU.mult,
                    ALU.mult)
                stt(rcol(5 + 2 * w_i), qa, K(KI_ONE), qb, ALU.mult, ALU.add)

            # --- layer-1 output state built directly as the rank-1
            # product f3 (x) f2 (x) f1, f_s = G_s (1,-i) = the fused
            # RX+RY wire state (phase included): f(0) = cw cx + i sw sx,
            # f(1) = sw cx - i cw sx. Every component and product is a
            # per-partition scalar -> 1-col ops, pipeline-free. ---
            for s in (1, 2, 3):
                base = 17 + 3 * s  # 20, 23, 26
                stt(prc(base), sac(W + s), K(KI_ONE), sac(s),
                    ALU.mult, ALU.mult)               # im0 = sw sx
                stt(prc(base + 1), sac(W + s), K(KI_ONE), csc(s),
                    ALU.mult, ALU.mult)               # re1 = sw cx
                tsm(prc(base + 2), prc(W + s), K(KI_NEGONE))  # im1 = -cw sx

            def fc(s, b):  # (re, im) PR col indices of f_s(b)
                base = 17 + 3 * s
                return (s, base) if b == 0 else (base + 1, base + 2)

            TMP = 37
            for b2 in (0, 1):  # S2 = f2 (x) f1 into PR[29:37]
                for b1 in (0, 1):
                    r2, i2 = fc(2, b2)
                    r1, i1 = fc(1, b1)
                    c = 29 + 2 * (2 * b2 + b1)
                    tsm(prc(TMP), prc(i2), prc(i1))
                    stt(prc(c), prc(r1), prc(r2), prc(TMP),
                        ALU.mult, ALU.subtract)
                    tsm(prc(TMP + 1), prc(i2), prc(r1))
                    stt(prc(c + 1), prc(i1), prc(r2), prc(TMP + 1),
                        ALU.mult, ALU.add)
            for b3 in (0, 1):  # st32 = f3 (x) S2, both v0 blocks
                r3, i3 = fc(3, b3)
                for b2 in (0, 1):
                    for b1 in (0, 1):
                        c = 29 + 2 * (2 * b2 + b1)
                        o = 2 * b1 + 4 * b2 + 16 * b3
                        tsm(prc(TMP), prc(i3), prc(c + 1))
                        tsm(prc(TMP + 1), prc(i3), prc(c))
                        for v in (0, 1):
                            stt(ST[0:NROWS, o + 8 * v:o + 8 * v + 1],
                                prc(c), prc(r3), prc(TMP),
                                ALU.mult, ALU.subtract)
                            stt(ST[0:NROWS, o + 8 * v + 1:o + 8 * v + 2],
                                prc(c + 1), prc(r3), prc(TMP + 1),
                                ALU.mult, ALU.add)

            # --- layer-2 conjugated RYs, all on the 32-col state ---
            t32v = t32.rearrange("p (a b) -> p a b", a=NST, b=2)
            st32v = st32.rearrange("p (a b) -> p a b", a=NST, b=2)
            # j=2 (bit b2)
            stt(t32v, st32v, sac(11), sgn(2), ALU.mult, ALU.mult)
            a0 = _bv(st32, NST2, 1, 0)
            a1 = _bv(st32, NST2, 1, 1)
            t0 = _bv(t32, NST2, 1, 0)
            t1 = _bv(t32, NST2, 1, 1)
            stt(a0, a0, csc(11), t1, ALU.mult, ALU.subtract)
            stt(a1, a1, csc(11), t0, ALU.mult, ALU.add)
            # j=3 (top bit b3): 2-op reversed-view form
            stt(t32v, st32v, sac(12), sgn(3), ALU.mult, ALU.mult)
            sv = st32.rearrange("p (c m) -> p c m", c=2, m=NST)
            tsw = t32.rearrange("p (c m) -> p c m",
                                c=2, m=NST)[:, ::-1, :]
            stt(sv, sv, csc(12), tsw, ALU.mult, ALU.add)
            # j=1 (bit b1)
            stt(t32v, st32v, sac(10), sgn(1), ALU.mult, ALU.mult)
            a0 = _bv(st32, NST2, 0, 0)
            a1 = _bv(st32, NST2, 0, 1)
            t0 = _bv(t32, NST2, 0, 0)
            t1 = _bv(t32, NST2, 0, 1)
            stt(a0, a0, csc(10), t1, ALU.mult, ALU.subtract)
            stt(a1, a1, csc(10), t0, ALU.mult, ALU.add)

            # --- measurement on bit b2; per-v0 accumulations ---
            A = _bv(st32, NST2, 1, 0)
            B = _bv(st32, NST2, 1, 1)
            TAv = _bv(t32, NST2, 1, 0)
            tsm(TAv, B, prc(10))
            stt(TAv, A, prc(11), TAv, ALU.mult, ALU.subtract)

            def vslice(ap, v, b):  # (b3, v0, b2, b1, ri) -> fix v0, b2
                w = ap.rearrange("p (a v c m) -> p a v c m", a=2, v=2,
                                 c=2, m=4)
                return w[:, :, v, b, :]

            sA = SCR[0:NROWS, 0:NST2]
            last = None
            for v in (0, 1):
                stt(vslice(sA, v, 0), vslice(T[0:NROWS, 0:NST2], v, 0),
                    K(KI_ONE), vslice(st32, v, 0), ALU.mult, ALU.mult,
                    accum_out=rcol(2 * v))
                last = stt(
                    vslice(sA, v, 1), vslice(st32, v, 1), prc(11),
                    vslice(st32, v, 1), ALU.mult, ALU.mult,
                    accum_out=rcol(2 * v + 1))
            last.then_inc(dve_sem, 1)

    _strip_barriers(nc)
    import bass_rust
    from concourse.hw_specs import get_activation_tables
    bass_rust.insert_act_table_loads(
        nc, list(get_activation_tables(nc.m.arch).items()))
    return nc


def _strip_barriers(nc):
    """Drop the auto-emitted prologue (const-AP memsets + all-engine
    barrier) and the epilogue barrier (the SP wait_ge(dma_sem, 32)
    orders program end after the output DMA lands)."""
    for bb in nc.m.functions[0].blocks:
        insts = bb.instructions
        keep = [i for i in insts
                if i.__class__.__name__ not in ("InstMemset", "InstDrain")
                and not (i.__class__.__name__ == "InstEventSemaphore"
                         and str(getattr(i, "name", "")).startswith(
                             "barrier_"))]
        if len(keep) != len(insts):
            insts[:] = keep


_NC_CACHE = None


def _get_nc():
    global _NC_CACHE
    if _NC_CACHE is None:
        _NC_CACHE = _build_nc()
    return _NC_CACHE


def _in_maps(x, params):
    A = _angle_table(x, params)  # [BATCH, NQ, NANGA]
    row_lo, row_hi = _const_rows()
    maps = []
    for c in range(NCORES):
        blk = np.zeros((NROWS, CC), np.float32)
        a = A[c * SPB:(c + 1) * SPB].reshape(ROWS, NANGA)
        blk[0:ROWS, 0:NANGA] = a
        blk[HI:NROWS, 0:NANGA] = a
        blk[0:ROWS, C_K:CC] = row_lo
        blk[HI:NROWS, C_K:CC] = row_hi
        maps.append({"inp": np.ascontiguousarray(blk)})
    return maps


def _run(x, params, trace=False):
    x = np.ascontiguousarray(np.asarray(x, np.float32))
    params = np.ascontiguousarray(np.asarray(params, np.float32))
    res = run_bass_kernel_spmd(
        _get_nc(), _in_maps(x, params), list(range(NCORES)), trace=trace)
    outs = []
    for c in range(NCORES):
        r = res.results[c]["outp"].reshape(NROWS, NRES)
        # per-row v0-weighted branch value, then v4-weighted row combine
        rowval = r[:, 4] * (r[:, 0] - r[:, 1]) + r[:, 5] * (r[:, 2] - r[:, 3])
        v = (r[0:ROWS, 6] * rowval[0:ROWS]
             + r[HI:NROWS, 7] * rowval[HI:NROWS])
        outs.append(-0.5 * v.reshape(SPB, NQ))
    return np.concatenate(outs, axis=0).astype(np.float32), res


def kernel(x, params):
    out, _ = _run(x, params)
    return out


# revision 41
# speedup vs baseline: 1.0016x; 1.0016x over previous
"""Trainium2 Bass kernel for nn_EstimatorQNN (18-qubit QNN, batch 16), v4.

Math: each <Z_i> is an exact 5-qubit light-cone sim over wires
{i-2..i+2}. v2 reductions (fused RX+RY layer-1 gates, CZ-conjugation
sigma masks, chi/gauge cancellation, layer-3 fold into the
measurement) still apply; v4 adds the CONTROL-BIT FACTORIZATION:

  Window wires i-2 and i+2 (bits 0 and 4) are diagonal controls for
  the whole circuit after layer 1 - they only enter through the
  CZ-conjugation signs sigma_1 = (-1)^(b0+b2), sigma_3 = (-1)^(b2+b4).
  Hence EXACTLY
      E = sum_{b0,b4} |f0(b0)|^2 |f4(b4)|^2 E(b0,b4),
  with f_s = G_s (1, -i) the post-layer-1 single-wire amplitudes. The
  simulated state shrinks to bits 1..3 (8 amps, 16 re/im cols), layer-1
  needs only 3 gates (on half-width ops), and the b0/b4 gates become
  per-branch weights shipped to the host (computed on-device as free
  1-col ops; the host does only the 4-term weighted average).

Layout per core: 2 samples x 18 windows = 36 sims; rows r (v4=0) and
64+r (v4=1), dead band 36-63 (partition bases must be 0/32/64/96).
State cols pre-replicate: 16 = (b3, b2, b1, ri) outer->inner; after
the v0 replication: 32 = (b3, v0, b2, b1, ri), so b3 stays the GLOBAL
top bit and layer-2 slot 3 keeps its 2-op reversed-view form (3D AP
limit: <=2 free dims).

Trig: ONE Activation op produces both tables - input angle block is
[a | a - pi] so Sin(0.5 in + pi/2) = cos(a/2) | sin(a/2) (table-exact;
rel err ~9e-5 = the complex64 reference's own noise floor).

Layer-2 ordering: slots 2 and 3 run BEFORE the v0 replication (their
conjugation signs don't involve v0 and the rotations commute), on
half-width operands; only slot 1 runs on the replicated state.

v5: the post-layer-1 state is a RANK-1 PRODUCT f3 (x) f2 (x) f1 with
f_s = G_s (1,-i) (phase included), and every component is a
per-partition scalar already on hand (cw*cx etc). The state is
therefore CONSTRUCTED directly - the textbook product-state build -
as 1-col scalar ops (free, pipelined), writing both v0 blocks of the
32-col state; the 12 wide layer-1 gate ops, the phase-table input and
the 2 replication ops all disappear. Only the three layer-2 rotations
(non-product) and the measurement remain as wide ops.

Timing structure (CoreSim, 5014ns total): startup 500 + input-DMA sem
~950 + ACT op -> DVE chain from 1591: ~60 free 1-col scalar ops
(trig products, w3 folds, branch weights, product-state build) then
14 serial wide ops (layer-2 j=2: 3, j=3: 2, j=1: 3, measurement: 6)
at the 61 + W_out/2 ns width law; sigma masks arrive via Pool piece 2
(first consumed by the j=2 cross-term op); then the fixed out-DMA
tail (~470 dispatch + 1717 DMACopy visit).

The Bass-emitted prologue (const-AP memsets + all-engine barrier) and
epilogue barrier are stripped post-build; program end is ordered after
the output DMA by an SP wait on its completion semaphore (without any
ordering the out-DMA races program end on real HW).
"""

import sys

sys.path.insert(0, "/opt/trn_rl_repo")

import numpy as np

import concourse.bass as bass
import concourse.mybir as mybir
from concourse.bass_utils import run_bass_kernel_spmd

NQ = 18
BATCH = 16
NCORES = 8
SPB = BATCH // NCORES  # samples per core
ROWS = SPB * NQ  # 36 sims per core (rows 0..35 and 64..99)
HI = 64  # partition base of the v4=1 row group
NROWS = HI + ROWS  # 100 partitions used
W = 5  # window width
NA8 = 8  # branch amplitudes (b1..b3)
NST = 2 * NA8  # 16 state cols pre-replicate
NST2 = 2 * NST  # 32 cols after v0 replication
NANG = 14  # angle cols: 5 x | 5 w1 | 3 w2(slots 1-3) | 1 w3
NANGA = 2 * NANG  # input angle block: [a | a - pi]
NK = 12  # const-scalar cols
NRES = 12  # raw moments SA2/SAB/SB2 per v0, weights, w3 coefficients
# input cols: [angles(28) | consts(12) || phase(16) | sgn1|sgn2|sgn3(16 ea)]
C_ANG = 0
C_K = NANGA
C_SG = NANGA + NK  # masks ride the single SP input DMA too
CC = C_SG + 3 * NST  # all masks 16-wide (state is built replicated)
C_AK = CC  # one flat input DMA carries everything

F32 = mybir.dt.float32
ALU = mybir.AluOpType

HALFPI = 1.5707963267948966
KVALS = [0.0, 0.0, HALFPI, 0.0, 0.0, 0.0, 1.0, -8.0, -4.0, 2.0, -1.0, 0.0]
(_KI_A, _KI_B, KI_HPI, _KI_CA, _KI_CB, _KI_C2, KI_ONE, KI_NEG8,
 KI_NEG4, KI_TWO, KI_NEGONE, _KI_PAD) = range(NK)


def _const_rows() -> tuple[np.ndarray, np.ndarray]:
    """(row_lo, row_hi): cols C_K..CC for the v4=0 / v4=1 row groups."""
    # phase over branch bits: col = ri + 2 b1 + 4 b2 + 8 b3
    a = np.arange(NA8)
    b1 = a & 1
    b2 = (a >> 1) & 1
    b3 = (a >> 2) & 1
    # masks over post-replication amps: idx = b1 + 2 b2 + 4 v0 + 8 b3
    m = np.arange(16)
    mb1 = m & 1
    mb2 = (m >> 1) & 1
    mv0 = (m >> 2) & 1
    mb3 = (m >> 3) & 1
    sgn1 = (-1.0) ** (mv0 + mb2)
    sgn2 = (-1.0) ** (mb1 + mb3)
    out = []
    for v4 in (0, 1):
        sgn3 = (1.0 - 2.0 * mb3) * ((-1.0) ** (mb2 + v4))
        out.append(np.concatenate(
            [KVALS, sgn1, sgn2, sgn3]).astype(np.float32))
    return out[0], out[1]


def _angle_table(x: np.ndarray, params: np.ndarray) -> np.ndarray:
    """[BATCH, NQ, NANGA] per-sim angles (0 for padded window slots)."""
    w1 = params[NQ:2 * NQ]
    w2 = params[2 * NQ:3 * NQ]
    w3 = params[3 * NQ:]
    A = np.zeros((BATCH, NQ, NANGA), np.float32)
    for i in range(NQ):
        for k in range(W):
            j = i - 2 + k
            if 0 <= j < NQ:
                A[:, i, k] = x[:, j]
                A[:, i, W + k] = w1[j]
        for m in range(3):  # L2 slots 1,2,3 -> wires i-1, i, i+1
            j = i - 1 + m
            if 0 <= j < NQ:
                A[:, i, 2 * W + m] = w2[j]
        A[:, i, 13] = w3[i]
    # cols 14..27: a - pi -> one ACT Sin op yields cos(a/2) AND sin(a/2)
    A[:, :, NANG:] = A[:, :, :NANG] - np.pi
    return A


def _bv(ap, ncols: int, k: int, b: int):
    """Bit-k == b view of an [p, ncols] re/im-interleaved state, bit
    order (.., k+1, k, .., ri): free dims [hi, lo] after slicing."""
    lo = 2 << k
    hi = ncols // (2 * lo)
    v = ap.rearrange("p (h c m) -> p h c m", h=hi, c=2, m=lo)
    return v[:, :, b, :]


def _build_nc(detect_races: bool = True) -> bass.Bass:
    nc = bass.Bass(detect_race_conditions=detect_races)
    inp = nc.dram_tensor("inp", [NROWS, CC], F32, kind="ExternalInput")
    outp = nc.dram_tensor("outp", [NROWS, NRES], F32, kind="ExternalOutput")

    with (
        nc.sbuf_tensor([128, CC], F32) as IN,
        nc.sbuf_tensor([128, 2 * NANG], F32) as TRG,  # cos | sin
        nc.sbuf_tensor([128, 40], F32) as PR,
        nc.sbuf_tensor([128, NST], F32) as ST16,
        nc.sbuf_tensor([128, NST2], F32) as ST,  # replicated state
        nc.sbuf_tensor([128, NST2], F32) as T,
        nc.sbuf_tensor([128, NST2], F32) as SCR,
        nc.sbuf_tensor([128, NRES], F32) as RES,
        nc.semaphore() as act_sem,
        nc.semaphore() as dma_sem,
        nc.semaphore() as dve_sem,
        nc.Block() as block,
    ):
        ang = IN[0:NROWS, C_ANG:C_ANG + NANGA]

        def K(i):
            return IN[0:NROWS, C_K + i:C_K + i + 1]

        st16 = ST16[0:NROWS, 0:NST]  # built by the rank-1 product
        st32 = ST[0:NROWS, 0:NST2]

        def sgn(j):  # layer-2 sign mask for slot j (re/im via bc)
            c0 = C_SG + (j - 1) * NST
            v = IN[0:NROWS, c0:c0 + NST].rearrange(
                "p (a b) -> p a b", a=NST, b=1)
            return v.broadcast_to([NROWS, NST, 2])

        trg = TRG[0:NROWS, 0:2 * NANG]
        t32 = T[0:NROWS, 0:NST2]

        def prc(i):
            return PR[0:NROWS, i:i + 1]

        def csc(i):
            return TRG[0:NROWS, i:i + 1]

        def sac(i):
            return TRG[0:NROWS, NANG + i:NANG + i + 1]

        def rcol(i):
            return RES[0:NROWS, i:i + 1]

        @block.sync
        def _(sync):
            sync.dma_start(
                out=IN[0:NROWS, 0:C_AK], in_=inp[:, 0:C_AK]).then_inc(
                dma_sem, 16)
            sync.dma_start(
                out=outp[:, :], in_=RES[0:NROWS, 0:NRES])._wait_ge(
                dve_sem, 1).then_inc(dma_sem, 16)
            sync.wait_ge(dma_sem, 32)

        @block.scalar
        def _(scalar):
            scalar.activation(
                trg, ang, mybir.ActivationFunctionType.Sin,
                bias=K(KI_HPI), scale=0.5)._wait_ge(dma_sem, 16).then_inc(
                act_sem, 1)

        @block.vector
        def _(vector):
            stt = vector.scalar_tensor_tensor
            ts = vector.tensor_scalar
            tsm = vector.tensor_scalar_mul

            # --- scalar products: PR[0:5]=cw1*cx, PR[5:10]=cw1*sx,
            # as ten 1-col ops (pipeline-hidden; on real HW they sit
            # inside the piece-2 DMA wait) ---
            first = True
            for k in range(W):
                op = vector.tensor_scalar_mul(prc(k), csc(W + k), csc(k))
                if first:
                    op._wait_ge(act_sem, 1)
                    first = False
                vector.tensor_scalar_mul(prc(W + k), csc(W + k), sac(k))
            # m2s = -4 sin(w3); n2c = -2 cos(w3) (1-col ops: free)
            stt(prc(10), sac(13), K(KI_NEG8), csc(13), ALU.mult, ALU.mult)
            stt(prc(11), csc(13), K(KI_NEG4), csc(13), ALU.mult, ALU.mult)
            vector.tensor_scalar_add(prc(11), prc(11), K(KI_TWO))

            # --- control-slot weights |f_s(b)|^2 into RES[4:8] (free
            # 1-col ops; f_s = G_s (1,-i), |f(0)|^2 = (cw cx)^2+(sw sx)^2,
            # |f(1)|^2 = (sw cx)^2 + (cw sx)^2, slots s=0 and 4) ---
            for w_i, s in ((0, 0), (1, 4)):
                u_ss = prc(12 + 2 * w_i)      # sw*sx
                u_sc = prc(13 + 2 * w_i)      # sw*cx
                stt(u_ss, sac(W + s), K(KI_ONE), sac(s), ALU.mult, ALU.mult)
                stt(u_sc, sac(W + s), K(KI_ONE), csc(s), ALU.mult, ALU.mult)
                qa = prc(16 + 2 * w_i)
                qb = prc(17 + 2 * w_i)
                stt(qa, prc(s), K(KI_ONE), prc(s), ALU.mult, ALU.mult)
                stt(qb, u_ss, K(KI_ONE), u_ss, ALU.mult, ALU.mult)
                stt(rcol(6 + 2 * w_i), qa, K(KI_ONE), qb, ALU.mult, ALU.add)
                stt(qa, u_sc, K(KI_ONE), u_sc, ALU.mult, ALU.mult)
                stt(qb, prc(W + s), K(KI_ONE), prc(W + s), ALU.mult,
                    ALU.mult)
                stt(rcol(7 + 2 * w_i), qa, K(KI_ONE), qb, ALU.mult, ALU.add)

            # --- layer-1 output state built directly as the rank-1
            # product f3 (x) f2 (x) f1, f_s = G_s (1,-i) = the fused
            # RX+RY wire state (phase included): f(0) = cw cx + i sw sx,
            # f(1) = sw cx - i cw sx. Every component and product is a
            # per-partition scalar -> 1-col ops, pipeline-free. ---
            for s in (1, 2, 3):
                base = 17 + 3 * s  # 20, 23, 26
                stt(prc(base), sac(W + s), K(KI_ONE), sac(s),
                    ALU.mult, ALU.mult)               # im0 = sw sx
                stt(prc(base + 1), sac(W + s), K(KI_ONE), csc(s),
                    ALU.mult, ALU.mult)               # re1 = sw cx
                tsm(prc(base + 2), prc(W + s), K(KI_NEGONE))  # im1 = -cw sx

            def fc(s, b):  # (re, im) PR col indices of f_s(b)
                base = 17 + 3 * s
                return (s, base) if b == 0 else (base + 1, base + 2)

            TMP = 37
            for b2 in (0, 1):  # S2 = f2 (x) f1 into PR[29:37]
                for b1 in (0, 1):
                    r2, i2 = fc(2, b2)
                    r1, i1 = fc(1, b1)
                    c = 29 + 2 * (2 * b2 + b1)
                    tsm(prc(TMP), prc(i2), prc(i1))
                    stt(prc(c), prc(r1), prc(r2), prc(TMP),
                        ALU.mult, ALU.subtract)
                    tsm(prc(TMP + 1), prc(i2), prc(r1))
                    stt(prc(c + 1), prc(i1), prc(r2), prc(TMP + 1),
                        ALU.mult, ALU.add)
            for b3 in (0, 1):  # st32 = f3 (x) S2, both v0 blocks
                r3, i3 = fc(3, b3)
                for b2 in (0, 1):
                    for b1 in (0, 1):
                        c = 29 + 2 * (2 * b2 + b1)
                        o = 2 * b1 + 4 * b2 + 16 * b3
                        tsm(prc(TMP), prc(i3), prc(c + 1))
                        tsm(prc(TMP + 1), prc(i3), prc(c))
                        for v in (0, 1):
                            stt(ST[0:NROWS, o + 8 * v:o + 8 * v + 1],
                                prc(c), prc(r3), prc(TMP),
                                ALU.mult, ALU.subtract)
                            stt(ST[0:NROWS, o + 8 * v + 1:o + 8 * v + 2],
                                prc(c + 1), prc(r3), prc(TMP + 1),
                                ALU.mult, ALU.add)

            # --- layer-2 conjugated RYs, all on the 32-col state ---
            t32v = t32.rearrange("p (a b) -> p a b", a=NST, b=2)
            st32v = st32.rearrange("p (a b) -> p a b", a=NST, b=2)
            # j=2 (bit b2)
            stt(t32v, st32v, sac(11), sgn(2), ALU.mult, ALU.mult)
            a0 = _bv(st32, NST2, 1, 0)
            a1 = _bv(st32, NST2, 1, 1)
            t0 = _bv(t32, NST2, 1, 0)
            t1 = _bv(t32, NST2, 1, 1)
            stt(a0, a0, csc(11), t1, ALU.mult, ALU.subtract)
            stt(a1, a1, csc(11), t0, ALU.mult, ALU.add)
            # j=3 (top bit b3): 2-op reversed-view form
            stt(t32v, st32v, sac(12), sgn(3), ALU.mult, ALU.mult)
            sv = st32.rearrange("p (c m) -> p c m", c=2, m=NST)
            tsw = t32.rearrange("p (c m) -> p c m",
                                c=2, m=NST)[:, ::-1, :]
            stt(sv, sv, csc(12), tsw, ALU.mult, ALU.add)
            # j=1 (bit b1)
            stt(t32v, st32v, sac(10), sgn(1), ALU.mult, ALU.mult)
            a0 = _bv(st32, NST2, 0, 0)
            a1 = _bv(st32, NST2, 0, 1)
            t0 = _bv(t32, NST2, 0, 0)
            t1 = _bv(t32, NST2, 0, 1)
            stt(a0, a0, csc(10), t1, ALU.mult, ALU.subtract)
            stt(a1, a1, csc(10), t0, ALU.mult, ALU.add)

            # --- measurement on bit b2: raw moments SA2, SAB, SB2 per
            # v0 variant; the w3 double-angle coefficients ship to the
            # host as free column writes ---
            tsm(rcol(10), prc(10), K(KI_ONE))   # m2s = -4 sin(w3)
            tsm(rcol(11), prc(11), K(KI_ONE))   # n2c = -2 cos(w3)

            def vslice(ap, v, b):  # (b3, v0, b2, b1, ri) -> fix v0, b2
                w = ap.rearrange("p (a v c m) -> p a v c m", a=2, v=2,
                                 c=2, m=4)
                return w[:, :, v, b, :]

            sA = SCR[0:NROWS, 0:NST2]
            last = None
            for v in (0, 1):
                Av = vslice(st32, v, 0)
                Bv = vslice(st32, v, 1)
                stt(vslice(sA, v, 0), Av, K(KI_ONE), Av, ALU.mult,
                    ALU.mult, accum_out=rcol(3 * v))
                stt(vslice(T[0:NROWS, 0:NST2], v, 0), Av, K(KI_ONE), Bv,
                    ALU.mult, ALU.mult, accum_out=rcol(3 * v + 1))
                last = stt(
                    vslice(sA, v, 1), Bv, K(KI_ONE), Bv, ALU.mult,
                    ALU.mult, accum_out=rcol(3 * v + 2))
            last.then_inc(dve_sem, 1)

    _strip_barriers(nc)
    import bass_rust
    from concourse.hw_specs import get_activation_tables
    bass_rust.insert_act_table_loads(
        nc, list(get_activation_tables(nc.m.arch).items()))
    return nc


def _strip_barriers(nc):
    """Drop the auto-emitted prologue (const-AP memsets + all-engine
    barrier) and the epilogue barrier (the SP wait_ge(dma_sem, 32)
    orders program end after the output DMA lands)."""
    for bb in nc.m.functions[0].blocks:
        insts = bb.instructions
        keep = [i for i in insts
                if i.__class__.__name__ not in ("InstMemset", "InstDrain")
                and not (i.__class__.__name__ == "InstEventSemaphore"
                         and str(getattr(i, "name", "")).startswith(
                             "barrier_"))]
        if len(keep) != len(insts):
            insts[:] = keep


_NC_CACHE = None


def _get_nc():
    global _NC_CACHE
    if _NC_CACHE is None:
        _NC_CACHE = _build_nc()
    return _NC_CACHE


def _in_maps(x, params):
    A = _angle_table(x, params)  # [BATCH, NQ, NANGA]
    row_lo, row_hi = _const_rows()
    maps = []
    for c in range(NCORES):
        blk = np.zeros((NROWS, CC), np.float32)
        a = A[c * SPB:(c + 1) * SPB].reshape(ROWS, NANGA)
        blk[0:ROWS, 0:NANGA] = a
        blk[HI:NROWS, 0:NANGA] = a
        blk[0:ROWS, C_K:CC] = row_lo
        blk[HI:NROWS, C_K:CC] = row_hi
        maps.append({"inp": np.ascontiguousarray(blk)})
    return maps


def _run(x, params, trace=False):
    x = np.ascontiguousarray(np.asarray(x, np.float32))
    params = np.ascontiguousarray(np.asarray(params, np.float32))
    res = run_bass_kernel_spmd(
        _get_nc(), _in_maps(x, params), list(range(NCORES)), trace=trace)
    outs = []
    for c in range(NCORES):
        r = res.results[c]["outp"].reshape(NROWS, NRES)
        # per-row v0-weighted branch value, then v4-weighted row combine
        vv = (r[:, 11:12] * (r[:, [0, 3]] - r[:, [2, 5]])
              - r[:, 10:11] * r[:, [1, 4]])  # per-variant n2c/m2s combine
        rowval = r[:, 6] * vv[:, 0] + r[:, 7] * vv[:, 1]
        v = (r[0:ROWS, 8] * rowval[0:ROWS]
             + r[HI:NROWS, 9] * rowval[HI:NROWS])
        outs.append(-0.5 * v.reshape(SPB, NQ))
    return np.concatenate(outs, axis=0).astype(np.float32), res


def kernel(x, params):
    out, _ = _run(x, params)
    return out


# revision 43
# speedup vs baseline: 1.0538x; 1.0521x over previous
"""Trainium2 Bass kernel for nn_EstimatorQNN (18-qubit QNN, batch 16), v4.

Math: each <Z_i> is an exact 5-qubit light-cone sim over wires
{i-2..i+2}. v2 reductions (fused RX+RY layer-1 gates, CZ-conjugation
sigma masks, chi/gauge cancellation, layer-3 fold into the
measurement) still apply; v4 adds the CONTROL-BIT FACTORIZATION:

  Window wires i-2 and i+2 (bits 0 and 4) are diagonal controls for
  the whole circuit after layer 1 - they only enter through the
  CZ-conjugation signs sigma_1 = (-1)^(b0+b2), sigma_3 = (-1)^(b2+b4).
  Hence EXACTLY
      E = sum_{b0,b4} |f0(b0)|^2 |f4(b4)|^2 E(b0,b4),
  with f_s = G_s (1, -i) the post-layer-1 single-wire amplitudes. The
  simulated state shrinks to bits 1..3 (8 amps, 16 re/im cols), layer-1
  needs only 3 gates (on half-width ops), and the b0/b4 gates become
  per-branch weights shipped to the host (computed on-device as free
  1-col ops; the host does only the 4-term weighted average).

Layout per core: 2 samples x 18 windows = 36 sims; rows r (v4=0) and
64+r (v4=1), dead band 36-63 (partition bases must be 0/32/64/96).
State cols pre-replicate: 16 = (b3, b2, b1, ri) outer->inner; after
the v0 replication: 32 = (b3, v0, b2, b1, ri), so b3 stays the GLOBAL
top bit and layer-2 slot 3 keeps its 2-op reversed-view form (3D AP
limit: <=2 free dims).

Trig: ONE Activation op produces both tables - input angle block is
[a | a - pi] so Sin(0.5 in + pi/2) = cos(a/2) | sin(a/2) (table-exact;
rel err ~9e-5 = the complex64 reference's own noise floor).

Layer-2 ordering: slots 2 and 3 run BEFORE the v0 replication (their
conjugation signs don't involve v0 and the rotations commute), on
half-width operands; only slot 1 runs on the replicated state.

v5: the post-layer-1 state is a RANK-1 PRODUCT f3 (x) f2 (x) f1 with
f_s = G_s (1,-i) (phase included), and every component is a
per-partition scalar already on hand (cw*cx etc). The state is
therefore CONSTRUCTED directly - the textbook product-state build -
as 1-col scalar ops (free, pipelined), writing both v0 blocks of the
32-col state; the 12 wide layer-1 gate ops, the phase-table input and
the 2 replication ops all disappear. Only the three layer-2 rotations
(non-product) and the measurement remain as wide ops.

v6: ONE flat input DMA carries angles + consts + masks (the Pool
piece is gone); the measurement ships raw moments SA2/SAB/SB2 per v0
variant plus the w3 coefficients, and the host does the 4-branch
weighted combine.

Timing structure (CoreSim, 5006ns total): startup 500 + input-DMA sem
~950 + ACT op -> DVE chain from 1591: ~100 free 1-col scalar ops
(trig products, w3 folds, branch weights, rank-1 product-state build)
then 14 serial wide ops (layer-2 rotations 3+2+3, measurement 6) at
the 61 + W_out/2 ns width law ending ~2790; then the fixed out-DMA
tail (~470 dispatch + 1717 DMACopy visit).

The Bass-emitted prologue (const-AP memsets + all-engine barrier) and
epilogue barrier are stripped post-build; program end is ordered after
the output DMA by an SP wait on its completion semaphore (without any
ordering the out-DMA races program end on real HW).
"""

import sys

sys.path.insert(0, "/opt/trn_rl_repo")

import numpy as np

import concourse.bass as bass
import concourse.mybir as mybir
from concourse.bass_utils import run_bass_kernel_spmd

NQ = 18
BATCH = 16
NCORES = 8
SPB = BATCH // NCORES  # samples per core
ROWS = SPB * NQ  # 36 sims per core (rows 0..35 and 64..99)
HI = 64  # partition base of the v4=1 row group
NROWS = HI + ROWS  # 100 partitions used
W = 5  # window width
NA8 = 8  # branch amplitudes (b1..b3)
NST = 2 * NA8  # 16 state cols pre-replicate
NST2 = 2 * NST  # 32 cols after v0 replication
NANG = 14  # angle cols: 5 x | 5 w1 | 3 w2(slots 1-3) | 1 w3
NANGA = 2 * NANG  # input angle block: [a | a - pi]
NK = 12  # const-scalar cols
NRES = 12  # raw moments SA2/SAB/SB2 per v0, weights, w3 coefficients
# input cols: [angles(28) | consts(12) || phase(16) | sgn1|sgn2|sgn3(16 ea)]
C_ANG = 0
C_K = NANGA
C_SG = NANGA + NK  # masks ride the single SP input DMA too
CC = C_SG + 3 * NST  # all masks 16-wide (state is built replicated)
C_AK = CC  # one flat input DMA carries everything

F32 = mybir.dt.float32
ALU = mybir.AluOpType

HALFPI = 1.5707963267948966
KVALS = [0.0, 0.0, HALFPI, 0.0, 0.0, 0.0, 1.0, -8.0, -4.0, 2.0, -1.0, 0.0]
(_KI_A, _KI_B, KI_HPI, _KI_CA, _KI_CB, _KI_C2, KI_ONE, KI_NEG8,
 KI_NEG4, KI_TWO, KI_NEGONE, _KI_PAD) = range(NK)


def _const_rows() -> tuple[np.ndarray, np.ndarray]:
    """(row_lo, row_hi): cols C_K..CC for the v4=0 / v4=1 row groups."""
    # phase over branch bits: col = ri + 2 b1 + 4 b2 + 8 b3
    a = np.arange(NA8)
    b1 = a & 1
    b2 = (a >> 1) & 1
    b3 = (a >> 2) & 1
    # masks over post-replication amps: idx = b1 + 2 b2 + 4 v0 + 8 b3
    m = np.arange(16)
    mb1 = m & 1
    mb2 = (m >> 1) & 1
    mv0 = (m >> 2) & 1
    mb3 = (m >> 3) & 1
    sgn1 = (-1.0) ** (mv0 + mb2)
    sgn2 = (-1.0) ** (mb1 + mb3)
    out = []
    for v4 in (0, 1):
        sgn3 = (1.0 - 2.0 * mb3) * ((-1.0) ** (mb2 + v4))
        out.append(np.concatenate(
            [KVALS, sgn1, sgn2, sgn3]).astype(np.float32))
    return out[0], out[1]


def _angle_table(x: np.ndarray, params: np.ndarray) -> np.ndarray:
    """[BATCH, NQ, NANGA] per-sim angles (0 for padded window slots)."""
    w1 = params[NQ:2 * NQ]
    w2 = params[2 * NQ:3 * NQ]
    w3 = params[3 * NQ:]
    A = np.zeros((BATCH, NQ, NANGA), np.float32)
    for i in range(NQ):
        for k in range(W):
            j = i - 2 + k
            if 0 <= j < NQ:
                A[:, i, k] = x[:, j]
                A[:, i, W + k] = w1[j]
        for m in range(3):  # L2 slots 1,2,3 -> wires i-1, i, i+1
            j = i - 1 + m
            if 0 <= j < NQ:
                A[:, i, 2 * W + m] = w2[j]
        A[:, i, 13] = w3[i]
    # cols 14..27: a - pi -> one ACT Sin op yields cos(a/2) AND sin(a/2)
    A[:, :, NANG:] = A[:, :, :NANG] - np.pi
    return A


def _bv(ap, ncols: int, k: int, b: int):
    """Bit-k == b view of an [p, ncols] re/im-interleaved state, bit
    order (.., k+1, k, .., ri): free dims [hi, lo] after slicing."""
    lo = 2 << k
    hi = ncols // (2 * lo)
    v = ap.rearrange("p (h c m) -> p h c m", h=hi, c=2, m=lo)
    return v[:, :, b, :]


def _build_nc(detect_races: bool = True) -> bass.Bass:
    nc = bass.Bass(detect_race_conditions=detect_races)
    inp = nc.dram_tensor("inp", [NROWS, CC], F32, kind="ExternalInput")
    outp = nc.dram_tensor("outp", [NROWS, NRES], F32, kind="ExternalOutput")

    with (
        nc.sbuf_tensor([128, CC], F32) as IN,
        nc.sbuf_tensor([128, 2 * NANG], F32) as TRG,  # cos | sin
        nc.sbuf_tensor([128, 64], F32) as PR,
        nc.sbuf_tensor([128, NST], F32) as ST16,
        nc.sbuf_tensor([128, NST2], F32) as ST,  # replicated state
        nc.sbuf_tensor([128, NST2], F32) as T,
        nc.sbuf_tensor([128, NST2], F32) as SCR,
        nc.sbuf_tensor([128, NRES], F32) as RES,
        nc.semaphore() as act_sem,
        nc.semaphore() as dma_sem,
        nc.semaphore() as dve_sem,
        nc.Block() as block,
    ):
        ang = IN[0:NROWS, C_ANG:C_ANG + NANGA]

        def K(i):
            return IN[0:NROWS, C_K + i:C_K + i + 1]

        st16 = ST16[0:NROWS, 0:NST]  # built by the rank-1 product
        st32 = ST[0:NROWS, 0:NST2]

        def sgn(j):  # layer-2 sign mask for slot j (re/im via bc)
            c0 = C_SG + (j - 1) * NST
            v = IN[0:NROWS, c0:c0 + NST].rearrange(
                "p (a b) -> p a b", a=NST, b=1)
            return v.broadcast_to([NROWS, NST, 2])

        trg = TRG[0:NROWS, 0:2 * NANG]
        t32 = T[0:NROWS, 0:NST2]

        def prc(i):
            return PR[0:NROWS, i:i + 1]

        def csc(i):
            return TRG[0:NROWS, i:i + 1]

        def sac(i):
            return TRG[0:NROWS, NANG + i:NANG + i + 1]

        def rcol(i):
            return RES[0:NROWS, i:i + 1]

        @block.sync
        def _(sync):
            sync.dma_start(
                out=IN[0:NROWS, 0:C_AK], in_=inp[:, 0:C_AK]).then_inc(
                dma_sem, 16)
            sync.dma_start(
                out=outp[:, :], in_=RES[0:NROWS, 0:NRES])._wait_ge(
                dve_sem, 1).then_inc(dma_sem, 16)
            sync.wait_ge(dma_sem, 32)

        @block.scalar
        def _(scalar):
            scalar.activation(
                trg, ang, mybir.ActivationFunctionType.Sin,
                bias=K(KI_HPI), scale=0.5)._wait_ge(dma_sem, 16).then_inc(
                act_sem, 1)

        @block.vector
        def _(vector):
            stt = vector.scalar_tensor_tensor
            ts = vector.tensor_scalar
            tsm = vector.tensor_scalar_mul

            # --- scalar products: PR[0:5]=cw1*cx, PR[5:10]=cw1*sx,
            # as ten 1-col ops (pipeline-hidden; on real HW they sit
            # inside the piece-2 DMA wait) ---
            first = True
            for k in range(W):
                op = vector.tensor_scalar_mul(prc(k), csc(W + k), csc(k))
                if first:
                    op._wait_ge(act_sem, 1)
                    first = False
                vector.tensor_scalar_mul(prc(W + k), csc(W + k), sac(k))
            # m2s = -4 sin(w3); n2c = -2 cos(w3) (1-col ops: free)
            stt(prc(10), sac(13), K(KI_NEG8), csc(13), ALU.mult, ALU.mult)
            stt(prc(11), csc(13), K(KI_NEG4), csc(13), ALU.mult, ALU.mult)
            vector.tensor_scalar_add(prc(11), prc(11), K(KI_TWO))

            # --- control-slot weights |f_s(b)|^2 into RES[4:8] (free
            # 1-col ops; f_s = G_s (1,-i), |f(0)|^2 = (cw cx)^2+(sw sx)^2,
            # |f(1)|^2 = (sw cx)^2 + (cw sx)^2, slots s=0 and 4) ---
            for w_i, s in ((0, 0), (1, 4)):
                u_ss = prc(12 + 2 * w_i)      # sw*sx
                u_sc = prc(13 + 2 * w_i)      # sw*cx
                stt(u_ss, sac(W + s), K(KI_ONE), sac(s), ALU.mult, ALU.mult)
                stt(u_sc, sac(W + s), K(KI_ONE), csc(s), ALU.mult, ALU.mult)
                qa = prc(16 + 2 * w_i)
                qb = prc(17 + 2 * w_i)
                stt(qa, prc(s), K(KI_ONE), prc(s), ALU.mult, ALU.mult)
                stt(qb, u_ss, K(KI_ONE), u_ss, ALU.mult, ALU.mult)
                stt(rcol(6 + 2 * w_i), qa, K(KI_ONE), qb, ALU.mult, ALU.add)
                stt(qa, u_sc, K(KI_ONE), u_sc, ALU.mult, ALU.mult)
                stt(qb, prc(W + s), K(KI_ONE), prc(W + s), ALU.mult,
                    ALU.mult)
                stt(rcol(7 + 2 * w_i), qa, K(KI_ONE), qb, ALU.mult, ALU.add)

            # --- layer-1 output state built directly as the rank-1
            # product f3 (x) f2 (x) f1, f_s = G_s (1,-i) = the fused
            # RX+RY wire state (phase included): f(0) = cw cx + i sw sx,
            # f(1) = sw cx - i cw sx. Every component and product is a
            # per-partition scalar -> 1-col ops, pipeline-free. ---
            for s in (1, 2, 3):
                base = 17 + 3 * s  # 20, 23, 26
                stt(prc(base), sac(W + s), K(KI_ONE), sac(s),
                    ALU.mult, ALU.mult)               # im0 = sw sx
                stt(prc(base + 1), sac(W + s), K(KI_ONE), csc(s),
                    ALU.mult, ALU.mult)               # re1 = sw cx
                tsm(prc(base + 2), prc(W + s), K(KI_NEGONE))  # im1 = -cw sx

            def fc(s, b):  # (re, im) PR col indices of f_s(b)
                base = 17 + 3 * s
                return (s, base) if b == 0 else (base + 1, base + 2)

            TMP = 37
            # layer-2 slot 2 folds into the build: applied first, its
            # sign sigma2 = (-1)^(b1+b3) only references other bits, so
            # the b2 factor is one of g+- = RY(+-w2) f2, element-chosen
            # at build time (state stays sector-wise rank-1).
            for gi, sg in ((0, 1.0), (1, -1.0)):  # g+ at 40, g- at 44
                for b in (0, 1):
                    rb, ib = fc(2, b)
                    ro, io = fc(2, 1 - b)
                    gc = 40 + 4 * gi + 2 * b
                    # g(b) = c2*f2(b) -+ sg*s2*f2(1-b)  (complex)
                    pm = ALU.subtract if (sg > 0) == (b == 0)                         else ALU.add
                    tsm(prc(TMP), sac(11), prc(ro))
                    stt(prc(gc), prc(rb), csc(11), prc(TMP),
                        ALU.mult, pm)
                    tsm(prc(TMP + 1), sac(11), prc(io))
                    stt(prc(gc + 1), prc(ib), csc(11), prc(TMP + 1),
                        ALU.mult, pm)

            for gi in (0, 1):  # S2(+-) = g(+-) (x) f1: cols 48+8gi
                for b2 in (0, 1):
                    for b1 in (0, 1):
                        r2, i2 = 40 + 4 * gi + 2 * b2, 41 + 4 * gi + 2 * b2
                        r1, i1 = fc(1, b1)
                        c = 48 + 8 * gi + 2 * (2 * b2 + b1)
                        tsm(prc(TMP), prc(i2), prc(i1))
                        stt(prc(c), prc(r1), prc(r2), prc(TMP),
                            ALU.mult, ALU.subtract)
                        tsm(prc(TMP + 1), prc(i2), prc(r1))
                        stt(prc(c + 1), prc(i1), prc(r2), prc(TMP + 1),
                            ALU.mult, ALU.add)
            for b3 in (0, 1):  # st32 = f3 (x) S2^sigma, both v0 blocks
                r3, i3 = fc(3, b3)
                for b2 in (0, 1):
                    for b1 in (0, 1):
                        gi = (b1 + b3) % 2  # sigma2 = +1 -> g+, -1 -> g-
                        c = 48 + 8 * gi + 2 * (2 * b2 + b1)
                        o = 2 * b1 + 4 * b2 + 16 * b3
                        tsm(prc(TMP), prc(i3), prc(c + 1))
                        tsm(prc(TMP + 1), prc(i3), prc(c))
                        for v in (0, 1):
                            stt(ST[0:NROWS, o + 8 * v:o + 8 * v + 1],
                                prc(c), prc(r3), prc(TMP),
                                ALU.mult, ALU.subtract)
                            stt(ST[0:NROWS, o + 8 * v + 1:o + 8 * v + 2],
                                prc(c + 1), prc(r3), prc(TMP + 1),
                                ALU.mult, ALU.add)

            # --- layer-2 conjugated RYs, all on the 32-col state ---
            t32v = t32.rearrange("p (a b) -> p a b", a=NST, b=2)
            st32v = st32.rearrange("p (a b) -> p a b", a=NST, b=2)
            # j=3 (top bit b3): 2-op reversed-view form
            stt(t32v, st32v, sac(12), sgn(3), ALU.mult, ALU.mult)
            sv = st32.rearrange("p (c m) -> p c m", c=2, m=NST)
            tsw = t32.rearrange("p (c m) -> p c m",
                                c=2, m=NST)[:, ::-1, :]
            stt(sv, sv, csc(12), tsw, ALU.mult, ALU.add)
            # j=1 (bit b1)
            stt(t32v, st32v, sac(10), sgn(1), ALU.mult, ALU.mult)
            a0 = _bv(st32, NST2, 0, 0)
            a1 = _bv(st32, NST2, 0, 1)
            t0 = _bv(t32, NST2, 0, 0)
            t1 = _bv(t32, NST2, 0, 1)
            stt(a0, a0, csc(10), t1, ALU.mult, ALU.subtract)
            stt(a1, a1, csc(10), t0, ALU.mult, ALU.add)

            # --- measurement on bit b2: raw moments SA2, SAB, SB2 per
            # v0 variant; the w3 double-angle coefficients ship to the
            # host as free column writes ---
            tsm(rcol(10), prc(10), K(KI_ONE))   # m2s = -4 sin(w3)
            tsm(rcol(11), prc(11), K(KI_ONE))   # n2c = -2 cos(w3)

            def vslice(ap, v, b):  # (b3, v0, b2, b1, ri) -> fix v0, b2
                w = ap.rearrange("p (a v c m) -> p a v c m", a=2, v=2,
                                 c=2, m=4)
                return w[:, :, v, b, :]

            sA = SCR[0:NROWS, 0:NST2]
            last = None
            for v in (0, 1):
                Av = vslice(st32, v, 0)
                Bv = vslice(st32, v, 1)
                stt(vslice(sA, v, 0), Av, K(KI_ONE), Av, ALU.mult,
                    ALU.mult, accum_out=rcol(3 * v))
                stt(vslice(T[0:NROWS, 0:NST2], v, 0), Av, K(KI_ONE), Bv,
                    ALU.mult, ALU.mult, accum_out=rcol(3 * v + 1))
                last = stt(
                    vslice(sA, v, 1), Bv, K(KI_ONE), Bv, ALU.mult,
                    ALU.mult, accum_out=rcol(3 * v + 2))
            last.then_inc(dve_sem, 1)

    _strip_barriers(nc)
    import bass_rust
    from concourse.hw_specs import get_activation_tables
    bass_rust.insert_act_table_loads(
        nc, list(get_activation_tables(nc.m.arch).items()))
    return nc


def _strip_barriers(nc):
    """Drop the auto-emitted prologue (const-AP memsets + all-engine
    barrier) and the epilogue barrier (the SP wait_ge(dma_sem, 32)
    orders program end after the output DMA lands)."""
    for bb in nc.m.functions[0].blocks:
        insts = bb.instructions
        keep = [i for i in insts
                if i.__class__.__name__ not in ("InstMemset", "InstDrain")
                and not (i.__class__.__name__ == "InstEventSemaphore"
                         and str(getattr(i, "name", "")).startswith(
                             "barrier_"))]
        if len(keep) != len(insts):
            insts[:] = keep


_NC_CACHE = None


def _get_nc():
    global _NC_CACHE
    if _NC_CACHE is None:
        _NC_CACHE = _build_nc()
    return _NC_CACHE


def _in_maps(x, params):
    A = _angle_table(x, params)  # [BATCH, NQ, NANGA]
    row_lo, row_hi = _const_rows()
    maps = []
    for c in range(NCORES):
        blk = np.zeros((NROWS, CC), np.float32)
        a = A[c * SPB:(c + 1) * SPB].reshape(ROWS, NANGA)
        blk[0:ROWS, 0:NANGA] = a
        blk[HI:NROWS, 0:NANGA] = a
        blk[0:ROWS, C_K:CC] = row_lo
        blk[HI:NROWS, C_K:CC] = row_hi
        maps.append({"inp": np.ascontiguousarray(blk)})
    return maps


def _run(x, params, trace=False):
    x = np.ascontiguousarray(np.asarray(x, np.float32))
    params = np.ascontiguousarray(np.asarray(params, np.float32))
    res = run_bass_kernel_spmd(
        _get_nc(), _in_maps(x, params), list(range(NCORES)), trace=trace)
    outs = []
    for c in range(NCORES):
        r = res.results[c]["outp"].reshape(NROWS, NRES)
        # per-row v0-weighted branch value, then v4-weighted row combine
        vv = (r[:, 11:12] * (r[:, [0, 3]] - r[:, [2, 5]])
              - r[:, 10:11] * r[:, [1, 4]])  # per-variant n2c/m2s combine
        rowval = r[:, 6] * vv[:, 0] + r[:, 7] * vv[:, 1]
        v = (r[0:ROWS, 8] * rowval[0:ROWS]
             + r[HI:NROWS, 9] * rowval[HI:NROWS])
        outs.append(-0.5 * v.reshape(SPB, NQ))
    return np.concatenate(outs, axis=0).astype(np.float32), res


def kernel(x, params):
    out, _ = _run(x, params)
    return out
